# revision 81
# baseline (speedup 1.0000x reference)
"""Trainium2 Bass kernel for 16-head MultiHeadAttention (B=2, T=2048, D=1024).

Sharding (8 NeuronCores): core c handles batch b = c//4 and head group
g = c%4 (heads 4g..4g+3).  Each core computes Q/K/V projections for its 4
heads, attention, and a partial output projection against its 256 rows of
W_O.  The host sums the 4 partials per batch and adds b_O (row-parallel TP;
the all-reduce is folded into the unshard step).

Device layout notes:
 - The host pre-transposes x to x^T [D, T] so the contraction dim (features)
   lands on SBUF partitions without any on-device transposes of x.
 - Attention is computed in the S^T = K @ Q^T orientation: the softmax
   denominator is then a partition-axis sum, which the PE produces for free
   via a ones-column appended to V (out = [V|1]^T @ P^T gives O^T rows 0..63
   and the denominator in row 64).
 - Per head pair (2 heads of 64), weights are stacked to fill 128 partitions.
 - Matmul operands are bf16 (fp32 PSUM accumulation).

Schedule notes (final):
 - V^T is computed directly on the PE (stationary = x^T key-chunk,
   moving = Wv) so no transposes / vn copies exist; bias lands via one
   strided DVE add against a broadcast bias tile.
 - All 8 attention stripes run as ONE software-pipelined stream: scores
   for global chunk g+3 are emitted (and exp'd) before the PV matmuls of
   chunk g, so the ACT exp stream crosses stripe boundaries without
   waiting on the ps_o drain.
 - Only a minimal prefix (K t-tile 0 and V^T/Q first halves for BOTH
   head pairs, interleaved per f-chunk to ride the DMA-paced start) runs
   before the stripes; every other projection and the output projection
   are paced filler thunks popped inside the stripes, sized against
   emission-order deadlines (a filler a prefetched score depends on must
   be emitted before it, or the in-order PE queue deadlocks).
 - Stripe tails: drain ps_o with 2 bf16 casts + denominator-row copies
   (split around the last PV), one wide reciprocal_approx_fast, bf16
   cast, GpSimd partition broadcasts, in-place multiplies.  The last
   stripe runs per-head chains with the copies on the idle ACT engine
   so the trailing outproj(3) starts ~3us sooner.
 - outproj psum lives on its own 1-bank pool (2 bufs); the final chunks
   alternate onto the freed score slots so matmuls don't serialize on
   store copies.  Output partials are stored bf16 (host sums in f32).
 - Input DMAs are sliced [128, 512-col] and issued on two sequencers
   (sync + scalar) in consumption order; output stores are split 4-way
   so SBUF staging slots free quickly.
"""

import os
import sys

import numpy as np

for _p in ("/opt/trn_rl_repo", "/root/.axon_site/_ro/trn_rl_repo"):
    if os.path.isdir(_p) and _p not in sys.path:
        sys.path.insert(0, _p)

import concourse.bass as bass
import concourse.mybir as mybir
import concourse.tile as tile
from concourse import bacc
from concourse.bass_utils import run_bass_kernel_spmd

F32 = mybir.dt.float32
BF16 = mybir.dt.bfloat16
AF = mybir.ActivationFunctionType

B, TQ, TK = 2, 2048, 2048
D = 1024          # model dim == x_to/x_from feature dim
H, DH = 16, 64
N_CORES = 8
HEADS_PER_CORE = 4   # one batch per core
HP = 2               # head pairs per core (2 heads of 64 stacked -> 128)

TT = 1024            # t-tile (bf16 moving free dim max)
N_TT = TQ // TT      # 2
N_SC = TK // 128     # 16 s-chunks
N_FC = D // 128      # 8 f-chunks
TA = 512             # attention stripe width (psum bank in f32)

DT = BF16

_CACHED = {}


def build_program():
    from collections import deque

    nc = bacc.Bacc(
        "TRN2", target_bir_lowering=False, debug=False, num_devices=N_CORES
    )

    xt_to = nc.dram_tensor("xt_to", [D, TQ], DT, kind="ExternalInput")
    xt_from = nc.dram_tensor("xt_from", [D, TK], DT, kind="ExternalInput")
    # weights arrive host-prearranged [partition, f-chunk, 256] so DMA
    # descriptors are contiguous 1KB-per-partition runs
    wq = nc.dram_tensor("wq", [128, N_FC, 256], DT, kind="ExternalInput")
    wk = nc.dram_tensor("wk", [128, N_FC, 256], DT, kind="ExternalInput")
    wv = nc.dram_tensor("wv", [128, N_FC, 256], DT, kind="ExternalInput")
    bq = nc.dram_tensor("bq", [128, 2], F32, kind="ExternalInput")
    bk = nc.dram_tensor("bk", [128, 2], F32, kind="ExternalInput")
    bvt = nc.dram_tensor("bvt", [1, 256], DT, kind="ExternalInput")
    wot = nc.dram_tensor("wot", [128, 2, 1024], DT, kind="ExternalInput")
    # bf16 partials: host sums 4 per batch in f32; rounding ~0.2% << gate
    out = nc.dram_tensor("out", [TQ, D], DT, kind="ExternalOutput")

    with tile.TileContext(nc) as tc:
        with (
            tc.tile_pool(name="wpool", bufs=1) as wpool,
            tc.tile_pool(name="actpool", bufs=1) as actpool,
            tc.tile_pool(name="ptpool", bufs=4) as ptpool,
            tc.tile_pool(name="misc", bufs=2) as misc,
            tc.tile_pool(name="psmm", bufs=2, space="PSUM") as psmm,
            tc.tile_pool(name="psacc", bufs=1, space="PSUM") as psacc,
            tc.tile_pool(name="psout", bufs=2, space="PSUM") as psout,
        ):
            # ---- constants & weights -------------------------------------
            wq_sb = wpool.tile([128, N_FC, 256], DT)
            wk_sb = wpool.tile([128, N_FC, 256], DT)
            wv_sb = wpool.tile([128, N_FC, 256], DT)
            bq_sb = wpool.tile([128, 2], F32)
            bk_sb = wpool.tile([128, 2], F32)
            bvt_sb = wpool.tile([1, 256], DT)
            # bv broadcast across rows, indexed [hp, h, d]
            bvbc = wpool.tile([128, HP, 2, 64], DT)
            wot_sb = wpool.tile([128, 2, 1024], DT)

            # x^T resident in SBUF
            xfr_sb = actpool.tile([128, N_FC, TK], DT, name="xfr_sb")
            xto_sb = actpool.tile([128, N_FC, TQ], DT, name="xto_sb")
            xt_to_r = xt_to.rearrange("(c p) t -> p c t", p=128)
            xt_from_r = xt_from.rearrange("(c p) t -> p c t", p=128)
            wq_r, wk_r, wv_r = wq, wk, wv

            # ---- input DMAs, sliced + ordered by first consumption -------
            # Three issue streams in parallel (~0.6us sequencer cost per
            # dma each): sync carries x_from, vector carries weights + x_to,
            # scalar (short queue — it has early transpose copies) carries
            # wq/wot.  Slicing spreads each tensor across many of the 16
            # DMA engines so the first consumers start a few us in.
            dma_s = nc.sync.dma_start
            dma_a = nc.scalar.dma_start
            for fcp in range(2):            # K weights first
                dma_a(
                    wk_sb[:, 4 * fcp : 4 * fcp + 4, :],
                    wk_r[:, 4 * fcp : 4 * fcp + 4, :],
                )
            # head-critical x slices alternate across both issue queues so
            # their ~0.6us/dma sequencer costs run in parallel: the prefix
            # consumes x_from cols 0:1024 then x_to cols 0:512.
            def dma_x(i, dst, src):
                (dma_s if i % 2 else dma_a)(dst, src)

            i = 0
            for fc in range(N_FC):          # x_from cols 0:512 (K h0, V^T)
                dma_x(i, xfr_sb[:, fc, 0:512], xt_from_r[:, fc, 0:512])
                i += 1
            dma_a(bk_sb[:], bk[:])
            dma_a(bvt_sb[:], bvt[:])
            dma_a(bq_sb[:], bq[:])
            for fc in range(N_FC):          # x_from cols 512:1024 (K h1)
                dma_x(i, xfr_sb[:, fc, 512:1024], xt_from_r[:, fc, 512:1024])
                i += 1
            for fcp in range(2):            # V weights
                dma_a(
                    wv_sb[:, 4 * fcp : 4 * fcp + 4, :],
                    wv_r[:, 4 * fcp : 4 * fcp + 4, :],
                )
            for fcp in range(2):            # Q weights
                dma_a(
                    wq_sb[:, 4 * fcp : 4 * fcp + 4, :],
                    wq_r[:, 4 * fcp : 4 * fcp + 4, :],
                )
            for fc in range(N_FC):          # x_to cols 0:512 (Q prefix half)
                dma_x(i, xto_sb[:, fc, 0:512], xt_to_r[:, fc, 0:512])
                i += 1
            # the rest in consumption order on sync
            for cs in range(2, 4):          # x_from cols 1024:2048 (tt1)
                sl = slice(cs * 512, cs * 512 + 512)
                for fc in range(N_FC):
                    dma_s(xfr_sb[:, fc, sl], xt_from_r[:, fc, sl])
            for cs in range(1, 4):          # x_to cols 512:2048
                sl = slice(cs * 512, cs * 512 + 512)
                for fc in range(N_FC):
                    dma_s(xto_sb[:, fc, sl], xt_to_r[:, fc, sl])
            for hp in range(HP):            # output proj weights (used late)
                dma_a(wot_sb[:, hp, :], wot[:, hp, :])

            # ---- persistent activations ----------------------------------
            qt_sb = [
                actpool.tile([128, TQ], DT, name=f"qt{hp}") for hp in range(HP)
            ]
            kt_sb = [
                actpool.tile([128, TK], DT, name=f"kt{hp}") for hp in range(HP)
            ]
            # V^T per s-chunk: [keys, head, dim | ones col]
            vn_sb = [
                actpool.tile([128, N_SC, 2, 65], DT, name=f"vn{hp}")
                for hp in range(HP)
            ]
            ot_sb = [
                actpool.tile([128, TQ], DT, name=f"ot{hp}") for hp in range(HP)
            ]
            # softmax denominator reciprocals on partition 0: (hp, h) at
            # offset (2*hp+h)*TQ; filled by reciprocal_approx_fast straight
            # from the PSUM denominator row.
            rec_all = actpool.tile([1, 4 * TQ], F32, name="rec_all")
            den_bf = actpool.tile([1, 4 * TQ], DT, name="den_bf")

            def emit_proj_pair(ps_name, w_sb, b_sb, x_sb, dst, tt):
                """Direct (prefix) projection tile for BOTH head pairs on
                the two psmm slots, interleaved per f-chunk so each
                arriving x slice feeds twice the matmul work while the
                prefix is DMA-paced."""
                ps = [
                    psmm.tile([128, TT], F32, tag="mm", name=f"{ps_name}{hp}")
                    for hp in range(HP)
                ]
                for half in range(2):
                    for fc in range(N_FC):
                        for hp in range(HP):
                            nc.tensor.matmul(
                                ps[hp][:, bass.ts(half, 512)],
                                w_sb[:, fc, bass.ts(hp, 128)],
                                x_sb[
                                    :, fc,
                                    tt * TT + half * 512 : tt * TT
                                    + half * 512
                                    + 512,
                                ],
                                start=(fc == 0),
                                stop=(fc == N_FC - 1),
                            )
                for hp in range(HP):
                    nc.vector.tensor_scalar_add(
                        dst[hp][:, bass.ts(tt, TT)],
                        ps[hp][:],
                        b_sb[:, hp : hp + 1],
                    )

            def emit_vn_ones(hp):
                nc.vector.memset(vn_sb[hp][:, :, :, 64], 1.0)

            def vt_thunks(hp, sc):
                """V^T for one 128-key chunk, computed directly on the PE:
                stationary = x^T chunk [feat, keys], moving = Wv slice
                [feat, 2*64 dims] -> psum [keys, 2, 64]; bias lands via one
                strided DVE add against the broadcast bias tile."""
                state = {}

                def mk_mm(g):
                    def mm(g=g):
                        if g == 0:
                            state["ps"] = psout.tile(
                                [128, 2, 64], F32, tag="out", name="ps_vt"
                            )
                        for fc in range(4 * g, 4 * g + 4):
                            nc.tensor.matmul(
                                state["ps"][:],
                                xfr_sb[:, fc, sc * 128 : sc * 128 + 128],
                                wv_sb[:, fc, bass.ts(hp, 128)],
                                start=(fc == 0),
                                stop=(fc == N_FC - 1),
                            )
                    return mm

                def add():
                    nc.vector.tensor_add(
                        vn_sb[hp][:, sc, :, 0:64],
                        state["ps"][:],
                        bvbc[:, hp, :, :],
                    )
                return [mk_mm(0), mk_mm(1), add]

            def proj_half_thunks(ps_name, w_sb, b_sb, x_sb, dst, hp, tt, half):
                """Filler-style K/Q projection: one 512-col half on a
                1-bank psout tile; 4 matmul thunks (2 f-chunks each) +
                copyback."""
                dsl = bass.ts(hp, 128)
                col0 = tt * TT + half * 512
                state = {}

                def mk_mm(fcp):
                    def mm(fcp=fcp):
                        if fcp == 0:
                            state["ps"] = psout.tile(
                                [128, 512], F32, tag="out", name=ps_name
                            )
                        for fc in (2 * fcp, 2 * fcp + 1):
                            nc.tensor.matmul(
                                state["ps"][:],
                                w_sb[:, fc, dsl],
                                x_sb[:, fc, col0 : col0 + 512],
                                start=(fc == 0),
                                stop=(fc == N_FC - 1),
                            )
                    return mm

                thunks = [mk_mm(fcp) for fcp in range(4)]

                def copyback():
                    nc.vector.tensor_scalar_add(
                        dst[hp][:, col0 : col0 + 512],
                        state["ps"][:],
                        b_sb[:, hp : hp + 1],
                    )
                thunks.append(copyback)
                return thunks, copyback

            STRIPES = [(hp, tta) for hp in range(HP) for tta in range(TQ // TA)]

            def emit_scores(si, sc):
                hp, tta = STRIPES[si]
                ps_s = psmm.tile([128, 1024], F32, tag="mm", name="ps_s")
                for h in range(2):
                    hb = 64 * h
                    nc.tensor.matmul(
                        ps_s[:, bass.ts(h, TA)],
                        kt_sb[hp][hb : hb + 64, bass.ts(sc, 128)],
                        qt_sb[hp][hb : hb + 64, bass.ts(tta, TA)],
                        start=True,
                        stop=True,
                    )
                pt = ptpool.tile([128, 1024], DT, tag="pt", name="pt")
                nc.scalar.activation(pt[:], ps_s[:], AF.Exp)
                return pt

            def norm_head(hp, tta, h, blk):
                """Reciprocal -> bf16 cast (ACT) -> broadcast -> multiply
                for one head; used for the last stripe where chain latency
                gates the trailing output projection."""
                o = blk + h * TA
                nc.vector.reciprocal_approx_fast(
                    out=rec_all[0:1, o : o + TA],
                    in_=rec_all[0:1, o : o + TA],
                )
                nc.scalar.activation(
                    den_bf[0:1, o : o + TA], rec_all[0:1, o : o + TA], AF.Copy
                )
                r_sb = misc.tile([128, TA], DT, tag="rsb", bufs=4, name="r_sb")
                nc.gpsimd.partition_broadcast(r_sb[:], den_bf[0:1, o : o + TA])
                hb = 64 * h
                nc.vector.tensor_mul(
                    ot_sb[hp][hb : hb + 64, bass.ts(tta, TA)],
                    ot_sb[hp][hb : hb + 64, bass.ts(tta, TA)],
                    r_sb[hb : hb + 64, :],
                )

            def emit_tail_h0(hp, tta, ps_o, last=False):
                """First half of the ps_o drain — emitted right after
                PV(15, h0) so it overlaps PV(15, h1).  For the last stripe
                the whole h0 normalize chain starts here too."""
                blk = ((hp * 4 + tta) * 2) * TA
                nc.vector.tensor_copy(
                    rec_all[0:1, blk : blk + TA], ps_o[64:65, 0:TA]
                )
                nc.vector.tensor_copy(
                    ot_sb[hp][0:64, bass.ts(tta, TA)], ps_o[0:64, 0:TA]
                )
                if last:
                    norm_head(hp, tta, 0, blk)

            def emit_tail(hp, tta, ps_o, last=False):
                """Rest of the drain + normalize ot in place.

                DVE: bf16 casts of the O rows, copies of the psum
                denominator row, one wide fast reciprocal, bf16 cast of it,
                2 multiplies.  GpSimd: 2 bf16 partition broadcasts (its
                only op type, so no Q7 library thrash).  For the last
                stripe the denominator copies run on the (now idle) scalar
                engine so the chain to outproj(3) is shorter.
                """
                blk = ((hp * 4 + tta) * 2) * TA
                if last:
                    nc.scalar.activation(
                        rec_all[0:1, blk + TA : blk + 2 * TA],
                        ps_o[64:65, TA : 2 * TA],
                        AF.Copy,
                    )
                    nc.vector.tensor_copy(
                        ot_sb[hp][64:128, bass.ts(tta, TA)],
                        ps_o[0:64, TA : 2 * TA],
                    )
                    norm_head(hp, tta, 1, blk)
                    return
                nc.vector.tensor_copy(
                    rec_all[0:1, blk + TA : blk + 2 * TA],
                    ps_o[64:65, TA : 2 * TA],
                )
                nc.vector.tensor_copy(
                    ot_sb[hp][64:128, bass.ts(tta, TA)],
                    ps_o[0:64, TA : 2 * TA],
                )
                nc.vector.reciprocal_approx_fast(
                    out=rec_all[0:1, blk : blk + 2 * TA],
                    in_=rec_all[0:1, blk : blk + 2 * TA],
                )
                nc.vector.tensor_copy(
                    den_bf[0:1, blk : blk + 2 * TA],
                    rec_all[0:1, blk : blk + 2 * TA],
                )
                for h in range(2):
                    r_sb = misc.tile(
                        [128, TA], DT, tag="rsb", bufs=4, name="r_sb"
                    )
                    nc.gpsimd.partition_broadcast(
                        r_sb[:], den_bf[0:1, blk + h * TA : blk + h * TA + TA]
                    )
                    hb = 64 * h
                    nc.vector.tensor_mul(
                        ot_sb[hp][hb : hb + 64, bass.ts(tta, TA)],
                        ot_sb[hp][hb : hb + 64, bass.ts(tta, TA)],
                        r_sb[hb : hb + 64, :],
                    )

            def emit_stripes(filler_pushes):
                """All 8 stripes as one pipelined stream: scores+exp run two
                s-chunks ahead of PV and cross stripe boundaries, so the ACT
                exp stream never waits on a ps_o drain.  filler_pushes maps
                stripe index -> (thunk list, start_sc, rate); thunks are
                popped inside that stripe at `rate` per iteration once
                sc >= start_sc."""
                from collections import deque

                fillers = deque()
                NS = len(STRIPES)
                pts = {}
                DEPTH = 3

                def prefetch(gi):
                    if gi < NS * N_SC:
                        pts[gi] = emit_scores(gi // N_SC, gi % N_SC)

                for gi in range(DEPTH):
                    prefetch(gi)
                for si in range(NS):
                    hp, tta = STRIPES[si]
                    extra = filler_pushes.get(si)
                    if extra:
                        fillers.extend(extra[0])
                        start_sc, rate = extra[1], extra[2]
                    else:
                        start_sc, rate = 0, 2.5
                    popped = 0
                    ps_o = psacc.tile([65, 1024], F32, tag="acc", name="ps_o")
                    for sc in range(N_SC):
                        # pop fillers FIRST so anything a prefetched score
                        # or PV depends on is already in the stream
                        if sc >= start_sc:
                            want = int((sc - start_sc + 1) * rate)
                            while popped < want and fillers:
                                fillers.popleft()()
                                popped += 1
                        pt = pts.pop(si * N_SC + sc)
                        for h in range(2):
                            nc.tensor.matmul(
                                ps_o[:, bass.ts(h, TA)],
                                vn_sb[hp][:, sc, h, 0:65],
                                pt[:, bass.ts(h, TA)],
                                start=(sc == 0),
                                stop=(sc == N_SC - 1),
                            )
                            if sc == N_SC - 1 and h == 0:
                                emit_tail_h0(
                                    hp, tta, ps_o, last=(si == NS - 1)
                                )
                        prefetch(si * N_SC + sc + DEPTH)
                    emit_tail(hp, tta, ps_o, last=(si == NS - 1))
                    # keep the PE warm across the boundary: a couple of
                    # filler thunks sit between this stripe's last PV and
                    # the next stripe's first (drain-gated) PV
                    for _ in range(2):
                        if fillers:
                            fillers.popleft()()
                            popped += 1
                while fillers:
                    fillers.popleft()()

            def emit_outproj_thunks(tta, store_on_act=False):
                """Output projection for one 512-query stripe: 4 t-chunks,
                each = 2 psum-half fills + 1 store thunk."""
                thunks = []
                for j in range(TA // 128):
                    tc_ = tta * (TA // 128) + j
                    tsl = bass.ts(tc_, 128)
                    state = {}
                    for half in range(2):
                        def mmf(half=half, state=state, tsl=tsl, j=j):
                            # final chunks alternate psum pools (scores are
                            # done, psmm is free) so chunk j+1's matmuls
                            # don't wait on chunk j's store copies
                            if store_on_act and j % 2:
                                ps = psmm.tile(
                                    [128, 512], F32, tag="mm", name="ps_out"
                                )
                            else:
                                ps = psout.tile(
                                    [128, 512], F32, tag="out", name="ps_out"
                                )
                            state[half] = ps
                            hsl = bass.ts(half, 512)
                            for hp in range(HP):
                                nc.tensor.matmul(
                                    ps[:],
                                    ot_sb[hp][:, tsl],
                                    wot_sb[:, hp, hsl],
                                    start=(hp == 0),
                                    stop=(hp == HP - 1),
                                )
                        thunks.append(mmf)

                    def store(state=state, tc_=tc_):
                        o_t = misc.tile(
                            [128, 1024], DT, tag="out", bufs=4, name="o_t"
                        )
                        for half in range(2):
                            # final stores split halves across ACT and DVE
                            # so the two copies run in parallel
                            if store_on_act and half == 0:
                                nc.scalar.activation(
                                    o_t[:, bass.ts(half, 512)],
                                    state[half][:],
                                    AF.Copy,
                                )
                            else:
                                nc.vector.tensor_copy(
                                    o_t[:, bass.ts(half, 512)], state[half][:]
                                )
                        # split the store so the o_t slot frees fast; the
                        # final stores alternate issue engines so the sync
                        # sequencer's ~0.6us/dma doesn't serialize the tail
                        for g in range(4):
                            r0 = tc_ * 128 + g * 32
                            eng = (
                                dma_a if store_on_act and g % 2 else dma_s
                            )
                            eng(
                                out[r0 : r0 + 32, :],
                                o_t[bass.ts(g, 32), :],
                            )
                    thunks.append(store)
                return thunks

            # ---- emission schedule ---------------------------------------
            # Prefix: the minimum hp0 projection work to start stripe
            # (0, 0) — K/V/ones for t-tile 0 plus the first Q half.
            # Everything else — K/V t-tile 1, remaining Q, all hp1
            # projections, and the output projections — runs as paced
            # fillers inside the stripes.  outproj(tta) is hosted in
            # stripe (hp1, tta+1); outproj(3) trails.
            # one-time broadcast of the V bias row (also warms the GpSimd
            # broadcast library well before the first stripe tail)
            nc.gpsimd.partition_broadcast(bvbc[:], bvt_sb[:])

            emit_vn_ones(0)
            emit_vn_ones(1)
            # Prefix: K t-tile 0 for BOTH head pairs (interleaved so the
            # DMA-paced stretch feeds double matmul work), V^T chunks 0-3
            # and the first Q half for both pairs.
            emit_proj_pair("ps_k", wk_sb, bk_sb, xfr_sb, kt_sb, 0)
            for sc in range(4):
                for hp in range(HP):
                    for f in vt_thunks(hp, sc):
                        f()
            for hp in range(HP):
                t, _ = proj_half_thunks(
                    "ps_q", wq_sb, bq_sb, xto_sb, qt_sb, hp, 0, 0
                )
                for f in t:
                    f()

            proj_rest = []

            def add_half(ps_name, w_sb, b_sb, x_sb, dst, hp, tt, half):
                t, _ = proj_half_thunks(
                    ps_name, w_sb, b_sb, x_sb, dst, hp, tt, half
                )
                proj_rest.extend(t)

            # ordered by the stripe-0 deadlines at rate 3.5 with
            # pops-before-PV (see pacing analysis in session notes)
            for sc in range(4, 8):
                proj_rest += vt_thunks(0, sc)
            add_half("ps_k", wk_sb, bk_sb, xfr_sb, kt_sb, 0, 1, 0)
            for sc in range(8, 12):
                proj_rest += vt_thunks(0, sc)
            add_half("ps_k", wk_sb, bk_sb, xfr_sb, kt_sb, 0, 1, 1)
            for sc in range(12, 14):
                proj_rest += vt_thunks(0, sc)
            add_half("ps_q", wq_sb, bq_sb, xto_sb, qt_sb, 0, 0, 1)
            for sc in range(14, 16):
                proj_rest += vt_thunks(0, sc)
            add_half("ps_q", wq_sb, bq_sb, xto_sb, qt_sb, 0, 1, 0)
            add_half("ps_q", wq_sb, bq_sb, xto_sb, qt_sb, 0, 1, 1)
            # remaining hp1 work (K tt0 / V^T 0-3 / Q h0 ran in the prefix)
            for tt_half in ((1, 0), (1, 1)):
                add_half(
                    "ps_k", wk_sb, bk_sb, xfr_sb, kt_sb, 1, tt_half[0],
                    tt_half[1],
                )
            for sc in range(4, 16):
                proj_rest += vt_thunks(1, sc)
            add_half("ps_q", wq_sb, bq_sb, xto_sb, qt_sb, 1, 0, 1)
            add_half("ps_q", wq_sb, bq_sb, xto_sb, qt_sb, 1, 1, 0)
            # Q(1,1) h1 is held out of proj_rest: its halves ride with the
            # outproj pushes in stripes 5/6 (their deadlines are that late)
            # so those stripes' boundary pops still find ready PE work.
            q11h1, _ = proj_half_thunks(
                "ps_q", wq_sb, bq_sb, xto_sb, qt_sb, 1, 1, 1
            )

            filler_pushes = {0: (proj_rest, 0, 3.5)}
            filler_pushes[5] = (
                q11h1 + emit_outproj_thunks(0), 5, 1.4,
            )
            filler_pushes[6] = (emit_outproj_thunks(1), 5, 1.0)
            filler_pushes[7] = (emit_outproj_thunks(2), 5, 1.0)
            emit_stripes(filler_pushes)
            for f in emit_outproj_thunks(3, store_on_act=True):
                f()

    nc.compile()
    return nc


def _prep_in_maps(x_to, x_from, Wq, bq, Wk, bk, Wv, bv, Wo):
    scale = 1.0 / np.sqrt(np.float32(DH))
    # [H, D, DH] -> [D, H*DH] with column h*DH+d
    wq_f = np.ascontiguousarray(Wq.transpose(1, 0, 2).reshape(D, H * DH)) * scale
    wk_f = np.ascontiguousarray(Wk.transpose(1, 0, 2).reshape(D, H * DH))
    wv_f = np.ascontiguousarray(Wv.transpose(1, 0, 2).reshape(D, H * DH))
    bq_f = bq.reshape(H * DH) * scale
    bk_f = bk.reshape(H * DH)
    bv_f = bv.reshape(H * DH)

    xt_to = np.ascontiguousarray(x_to.transpose(0, 2, 1))    # [B, D, TQ]
    xt_from = np.ascontiguousarray(x_from.transpose(0, 2, 1))

    def f32(a):
        return np.ascontiguousarray(a, dtype=np.float32)

    import ml_dtypes

    def fdt(a):
        return np.ascontiguousarray(a, dtype=ml_dtypes.bfloat16)

    in_maps = []
    for c in range(N_CORES):
        b, g = divmod(c, HEADS_PER_CORE)
        cs = slice(g * 256, (g + 1) * 256)
        in_maps.append(
            {
                "xt_to": fdt(xt_to[b]),
                "xt_from": fdt(xt_from[b]),
                # [1024, 256] -> [128 partitions, 8 f-chunks, 256] so each
                # partition's DMA reads are contiguous runs
                "wq": fdt(
                    wq_f[:, cs].reshape(8, 128, 256).transpose(1, 0, 2)
                ),
                "wk": fdt(
                    wk_f[:, cs].reshape(8, 128, 256).transpose(1, 0, 2)
                ),
                "wv": fdt(
                    wv_f[:, cs].reshape(8, 128, 256).transpose(1, 0, 2)
                ),
                # [256] -> [2 pairs, 128] -> [128, 2]
                "bq": f32(bq_f[cs].reshape(2, 128).T),
                "bk": f32(bk_f[cs].reshape(2, 128).T),
                "bvt": fdt(bv_f[cs].reshape(1, 256)),
                # Wo[:, cs].T = [256, 1024] -> [2, 128, 1024] -> [128, 2, 1024]
                "wot": fdt(
                    np.ascontiguousarray(Wo[:, cs].T)
                    .reshape(2, 128, 1024)
                    .transpose(1, 0, 2)
                ),
            }
        )
    return in_maps


LAST_EXEC_TIME_NS = None
LAST_TRACE = None


def kernel(x_to, x_from, Wq, bq, Wk, bk, Wv, bv, Wo, bo):
    global LAST_EXEC_TIME_NS, LAST_TRACE
    if "nc" not in _CACHED:
        _CACHED["nc"] = build_program()
    nc = _CACHED["nc"]

    in_maps = _prep_in_maps(
        np.asarray(x_to), np.asarray(x_from), np.asarray(Wq), np.asarray(bq),
        np.asarray(Wk), np.asarray(bk), np.asarray(Wv), np.asarray(bv),
        np.asarray(Wo),
    )
    res = run_bass_kernel_spmd(nc, in_maps, list(range(N_CORES)))
    LAST_EXEC_TIME_NS = res.exec_time_ns
    LAST_TRACE = res.instructions_and_trace

    out = np.zeros((B, TQ, D), dtype=np.float32)
    for c in range(N_CORES):
        out[c // HEADS_PER_CORE] += res.results[c]["out"].astype(np.float32)
    out += np.asarray(bo, dtype=np.float32)
    return out


# revision 83
# speedup vs baseline: 1.0069x; 1.0069x over previous
"""Trainium2 Bass kernel for 16-head MultiHeadAttention (B=2, T=2048, D=1024).

Sharding (8 NeuronCores): core c handles batch b = c//4 and head group
g = c%4 (heads 4g..4g+3).  Each core computes Q/K/V projections for its 4
heads, attention, and a partial output projection against its 256 rows of
W_O.  The host sums the 4 partials per batch and adds b_O (row-parallel TP;
the all-reduce is folded into the unshard step).

Device layout notes:
 - The host pre-transposes x to x^T [D, T] so the contraction dim (features)
   lands on SBUF partitions without any on-device transposes of x.
 - Attention is computed in the S^T = K @ Q^T orientation: the softmax
   denominator is then a partition-axis sum, which the PE produces for free
   via a ones-column appended to V (out = [V|1]^T @ P^T gives O^T rows 0..63
   and the denominator in row 64).
 - Per head pair (2 heads of 64), weights are stacked to fill 128 partitions.
 - Matmul operands are bf16 (fp32 PSUM accumulation).

Schedule notes (final):
 - V^T is computed directly on the PE (stationary = x^T key-chunk,
   moving = Wv) so no transposes / vn copies exist; bias lands via one
   strided DVE add against a broadcast bias tile.
 - All 8 attention stripes run as ONE software-pipelined stream: scores
   for global chunk g+3 are emitted (and exp'd) before the PV matmuls of
   chunk g, so the ACT exp stream crosses stripe boundaries without
   waiting on the ps_o drain.
 - Only a minimal prefix (K t-tile 0 and V^T/Q first halves for BOTH
   head pairs, interleaved per f-chunk to ride the DMA-paced start) runs
   before the stripes; every other projection and the output projection
   are paced filler thunks popped inside the stripes, sized against
   emission-order deadlines (a filler a prefetched score depends on must
   be emitted before it, or the in-order PE queue deadlocks).
 - Stripe tails: drain ps_o with 2 bf16 casts + denominator-row copies
   (split around the last PV), one wide reciprocal_approx_fast, bf16
   cast, GpSimd partition broadcasts, in-place multiplies.  The last
   stripe runs per-head chains with the copies on the idle ACT engine
   so the trailing outproj(3) starts ~3us sooner.
 - outproj psum lives on its own 1-bank pool (2 bufs); the final chunks
   alternate onto the freed score slots so matmuls don't serialize on
   store copies.  Output partials are stored bf16 (host sums in f32).
 - Input DMAs are sliced [128, 512-col] and issued on two sequencers
   (sync + scalar) in consumption order; output stores are split 4-way
   so SBUF staging slots free quickly.
"""

import os
import sys

import numpy as np

for _p in ("/opt/trn_rl_repo", "/root/.axon_site/_ro/trn_rl_repo"):
    if os.path.isdir(_p) and _p not in sys.path:
        sys.path.insert(0, _p)

import concourse.bass as bass
import concourse.mybir as mybir
import concourse.tile as tile
from concourse import bacc
from concourse.bass_utils import run_bass_kernel_spmd

F32 = mybir.dt.float32
BF16 = mybir.dt.bfloat16
AF = mybir.ActivationFunctionType

B, TQ, TK = 2, 2048, 2048
D = 1024          # model dim == x_to/x_from feature dim
H, DH = 16, 64
N_CORES = 8
HEADS_PER_CORE = 4   # one batch per core
HP = 2               # head pairs per core (2 heads of 64 stacked -> 128)

TT = 1024            # t-tile (bf16 moving free dim max)
N_TT = TQ // TT      # 2
N_SC = TK // 128     # 16 s-chunks
N_FC = D // 128      # 8 f-chunks
TA = 512             # attention stripe width (psum bank in f32)

DT = BF16

_CACHED = {}


def build_program():
    from collections import deque

    nc = bacc.Bacc(
        "TRN2", target_bir_lowering=False, debug=False, num_devices=N_CORES
    )

    xt_to = nc.dram_tensor("xt_to", [D, TQ], DT, kind="ExternalInput")
    xt_from = nc.dram_tensor("xt_from", [D, TK], DT, kind="ExternalInput")
    wq = nc.dram_tensor("wq", [D, 256], DT, kind="ExternalInput")
    wk = nc.dram_tensor("wk", [D, 256], DT, kind="ExternalInput")
    wv = nc.dram_tensor("wv", [D, 256], DT, kind="ExternalInput")
    bq = nc.dram_tensor("bq", [128, 2], F32, kind="ExternalInput")
    bk = nc.dram_tensor("bk", [128, 2], F32, kind="ExternalInput")
    bvt = nc.dram_tensor("bvt", [1, 256], DT, kind="ExternalInput")
    wot = nc.dram_tensor("wot", [128, 2, 1024], DT, kind="ExternalInput")
    # bf16 partials: host sums 4 per batch in f32; rounding ~0.2% << gate
    out = nc.dram_tensor("out", [TQ, D], DT, kind="ExternalOutput")

    with tile.TileContext(nc) as tc:
        with (
            tc.tile_pool(name="wpool", bufs=1) as wpool,
            tc.tile_pool(name="actpool", bufs=1) as actpool,
            tc.tile_pool(name="ptpool", bufs=4) as ptpool,
            tc.tile_pool(name="misc", bufs=2) as misc,
            tc.tile_pool(name="psmm", bufs=2, space="PSUM") as psmm,
            tc.tile_pool(name="psacc", bufs=1, space="PSUM") as psacc,
            tc.tile_pool(name="psout", bufs=2, space="PSUM") as psout,
        ):
            # ---- constants & weights -------------------------------------
            wq_sb = wpool.tile([128, N_FC, 256], DT)
            wk_sb = wpool.tile([128, N_FC, 256], DT)
            wv_sb = wpool.tile([128, N_FC, 256], DT)
            bq_sb = wpool.tile([128, 2], F32)
            bk_sb = wpool.tile([128, 2], F32)
            bvt_sb = wpool.tile([1, 256], DT)
            # bv broadcast across rows, indexed [hp, h, d]
            bvbc = wpool.tile([128, HP, 2, 64], DT)
            wot_sb = wpool.tile([128, 2, 1024], DT)

            # PE warmup: throwaway matmuls on a zeroed tile so the tensor
            # engine's p-state ramps to full clock while the first x
            # slices are still in flight; the real prefix then runs at
            # ~216ns/512cols instead of the cold 427-585ns.
            warm = wpool.tile([128, 512], DT, name="warm")
            nc.vector.memset(warm[:], 0.0)
            for _w in range(12):
                ps_w = psout.tile([128, 512], F32, tag="out", name="ps_w")
                nc.tensor.matmul(
                    ps_w[:], warm[:, 0:128], warm[:], start=True, stop=True
                )

            # x^T resident in SBUF
            xfr_sb = actpool.tile([128, N_FC, TK], DT, name="xfr_sb")
            xto_sb = actpool.tile([128, N_FC, TQ], DT, name="xto_sb")
            xt_to_r = xt_to.rearrange("(c p) t -> p c t", p=128)
            xt_from_r = xt_from.rearrange("(c p) t -> p c t", p=128)
            wq_r = wq.rearrange("(c p) d -> p c d", p=128)
            wk_r = wk.rearrange("(c p) d -> p c d", p=128)
            wv_r = wv.rearrange("(c p) d -> p c d", p=128)

            # ---- input DMAs, sliced + ordered by first consumption -------
            # Three issue streams in parallel (~0.6us sequencer cost per
            # dma each): sync carries x_from, vector carries weights + x_to,
            # scalar (short queue — it has early transpose copies) carries
            # wq/wot.  Slicing spreads each tensor across many of the 16
            # DMA engines so the first consumers start a few us in.
            dma_s = nc.sync.dma_start
            dma_a = nc.scalar.dma_start
            for fcp in range(2):            # K weights first
                dma_a(
                    wk_sb[:, 4 * fcp : 4 * fcp + 4, :],
                    wk_r[:, 4 * fcp : 4 * fcp + 4, :],
                )
            # head-critical x slices alternate across both issue queues so
            # their ~0.6us/dma sequencer costs run in parallel: the prefix
            # consumes x_from cols 0:1024 then x_to cols 0:512.
            def dma_x(i, dst, src):
                (dma_s if i % 2 else dma_a)(dst, src)

            i = 0
            for fc in range(N_FC):          # x_from cols 0:512 (K h0, V^T)
                dma_x(i, xfr_sb[:, fc, 0:512], xt_from_r[:, fc, 0:512])
                i += 1
            dma_a(bk_sb[:], bk[:])
            dma_a(bvt_sb[:], bvt[:])
            dma_a(bq_sb[:], bq[:])
            for fc in range(N_FC):          # x_from cols 512:1024 (K h1)
                dma_x(i, xfr_sb[:, fc, 512:1024], xt_from_r[:, fc, 512:1024])
                i += 1
            for fcp in range(2):            # V weights
                dma_a(
                    wv_sb[:, 4 * fcp : 4 * fcp + 4, :],
                    wv_r[:, 4 * fcp : 4 * fcp + 4, :],
                )
            for fcp in range(2):            # Q weights
                dma_a(
                    wq_sb[:, 4 * fcp : 4 * fcp + 4, :],
                    wq_r[:, 4 * fcp : 4 * fcp + 4, :],
                )
            for fc in range(N_FC):          # x_to cols 0:512 (Q prefix half)
                dma_x(i, xto_sb[:, fc, 0:512], xt_to_r[:, fc, 0:512])
                i += 1
            # the rest in consumption order on sync
            for cs in range(2, 4):          # x_from cols 1024:2048 (tt1)
                sl = slice(cs * 512, cs * 512 + 512)
                for fc in range(N_FC):
                    dma_s(xfr_sb[:, fc, sl], xt_from_r[:, fc, sl])
            for cs in range(1, 4):          # x_to cols 512:2048
                sl = slice(cs * 512, cs * 512 + 512)
                for fc in range(N_FC):
                    dma_s(xto_sb[:, fc, sl], xt_to_r[:, fc, sl])
            for hp in range(HP):            # output proj weights (used late)
                dma_a(wot_sb[:, hp, :], wot[:, hp, :])

            # ---- persistent activations ----------------------------------
            qt_sb = [
                actpool.tile([128, TQ], DT, name=f"qt{hp}") for hp in range(HP)
            ]
            kt_sb = [
                actpool.tile([128, TK], DT, name=f"kt{hp}") for hp in range(HP)
            ]
            # V^T per s-chunk: [keys, head, dim | ones col]
            vn_sb = [
                actpool.tile([128, N_SC, 2, 65], DT, name=f"vn{hp}")
                for hp in range(HP)
            ]
            ot_sb = [
                actpool.tile([128, TQ], DT, name=f"ot{hp}") for hp in range(HP)
            ]
            # softmax denominator reciprocals on partition 0: (hp, h) at
            # offset (2*hp+h)*TQ; filled by reciprocal_approx_fast straight
            # from the PSUM denominator row.
            rec_all = actpool.tile([1, 4 * TQ], F32, name="rec_all")
            den_bf = actpool.tile([1, 4 * TQ], DT, name="den_bf")

            def emit_proj_pair(ps_name, w_sb, b_sb, x_sb, dst, tt):
                """Direct (prefix) projection tile for BOTH head pairs on
                the two psmm slots, interleaved per f-chunk so each
                arriving x slice feeds twice the matmul work while the
                prefix is DMA-paced."""
                ps = [
                    psmm.tile([128, TT], F32, tag="mm", name=f"{ps_name}{hp}")
                    for hp in range(HP)
                ]
                for half in range(2):
                    for fc in range(N_FC):
                        for hp in range(HP):
                            nc.tensor.matmul(
                                ps[hp][:, bass.ts(half, 512)],
                                w_sb[:, fc, bass.ts(hp, 128)],
                                x_sb[
                                    :, fc,
                                    tt * TT + half * 512 : tt * TT
                                    + half * 512
                                    + 512,
                                ],
                                start=(fc == 0),
                                stop=(fc == N_FC - 1),
                            )
                for hp in range(HP):
                    nc.vector.tensor_scalar_add(
                        dst[hp][:, bass.ts(tt, TT)],
                        ps[hp][:],
                        b_sb[:, hp : hp + 1],
                    )

            def emit_vn_ones(hp):
                nc.vector.memset(vn_sb[hp][:, :, :, 64], 1.0)

            def vt_thunks(hp, sc):
                """V^T for one 128-key chunk, computed directly on the PE:
                stationary = x^T chunk [feat, keys], moving = Wv slice
                [feat, 2*64 dims] -> psum [keys, 2, 64]; bias lands via one
                strided DVE add against the broadcast bias tile."""
                state = {}

                def mk_mm(g):
                    def mm(g=g):
                        if g == 0:
                            state["ps"] = psout.tile(
                                [128, 2, 64], F32, tag="out", name="ps_vt"
                            )
                        for fc in range(4 * g, 4 * g + 4):
                            nc.tensor.matmul(
                                state["ps"][:],
                                xfr_sb[:, fc, sc * 128 : sc * 128 + 128],
                                wv_sb[:, fc, bass.ts(hp, 128)],
                                start=(fc == 0),
                                stop=(fc == N_FC - 1),
                            )
                    return mm

                def add():
                    nc.vector.tensor_add(
                        vn_sb[hp][:, sc, :, 0:64],
                        state["ps"][:],
                        bvbc[:, hp, :, :],
                    )
                return [mk_mm(0), mk_mm(1), add]

            def proj_half_thunks(ps_name, w_sb, b_sb, x_sb, dst, hp, tt, half):
                """Filler-style K/Q projection: one 512-col half on a
                1-bank psout tile; 4 matmul thunks (2 f-chunks each) +
                copyback."""
                dsl = bass.ts(hp, 128)
                col0 = tt * TT + half * 512
                state = {}

                def mk_mm(fcp):
                    def mm(fcp=fcp):
                        if fcp == 0:
                            state["ps"] = psout.tile(
                                [128, 512], F32, tag="out", name=ps_name
                            )
                        for fc in (2 * fcp, 2 * fcp + 1):
                            nc.tensor.matmul(
                                state["ps"][:],
                                w_sb[:, fc, dsl],
                                x_sb[:, fc, col0 : col0 + 512],
                                start=(fc == 0),
                                stop=(fc == N_FC - 1),
                            )
                    return mm

                thunks = [mk_mm(fcp) for fcp in range(4)]

                def copyback():
                    nc.vector.tensor_scalar_add(
                        dst[hp][:, col0 : col0 + 512],
                        state["ps"][:],
                        b_sb[:, hp : hp + 1],
                    )
                thunks.append(copyback)
                return thunks, copyback

            STRIPES = [(hp, tta) for hp in range(HP) for tta in range(TQ // TA)]

            def emit_scores(si, sc):
                hp, tta = STRIPES[si]
                ps_s = psmm.tile([128, 1024], F32, tag="mm", name="ps_s")
                for h in range(2):
                    hb = 64 * h
                    nc.tensor.matmul(
                        ps_s[:, bass.ts(h, TA)],
                        kt_sb[hp][hb : hb + 64, bass.ts(sc, 128)],
                        qt_sb[hp][hb : hb + 64, bass.ts(tta, TA)],
                        start=True,
                        stop=True,
                    )
                pt = ptpool.tile([128, 1024], DT, tag="pt", name="pt")
                nc.scalar.activation(pt[:], ps_s[:], AF.Exp)
                return pt

            def norm_head(hp, tta, h, blk):
                """Reciprocal -> bf16 cast (ACT) -> broadcast -> multiply
                for one head; used for the last stripe where chain latency
                gates the trailing output projection."""
                o = blk + h * TA
                nc.vector.reciprocal_approx_fast(
                    out=rec_all[0:1, o : o + TA],
                    in_=rec_all[0:1, o : o + TA],
                )
                nc.scalar.activation(
                    den_bf[0:1, o : o + TA], rec_all[0:1, o : o + TA], AF.Copy
                )
                r_sb = misc.tile([128, TA], DT, tag="rsb", bufs=4, name="r_sb")
                nc.gpsimd.partition_broadcast(r_sb[:], den_bf[0:1, o : o + TA])
                hb = 64 * h
                nc.vector.tensor_mul(
                    ot_sb[hp][hb : hb + 64, bass.ts(tta, TA)],
                    ot_sb[hp][hb : hb + 64, bass.ts(tta, TA)],
                    r_sb[hb : hb + 64, :],
                )

            def emit_tail_h0(hp, tta, ps_o, last=False):
                """First half of the ps_o drain — emitted right after
                PV(15, h0) so it overlaps PV(15, h1).  For the last stripe
                the whole h0 normalize chain starts here too."""
                blk = ((hp * 4 + tta) * 2) * TA
                nc.vector.tensor_copy(
                    rec_all[0:1, blk : blk + TA], ps_o[64:65, 0:TA]
                )
                nc.vector.tensor_copy(
                    ot_sb[hp][0:64, bass.ts(tta, TA)], ps_o[0:64, 0:TA]
                )
                if last:
                    norm_head(hp, tta, 0, blk)

            def emit_tail(hp, tta, ps_o, last=False):
                """Rest of the drain + normalize ot in place.

                DVE: bf16 casts of the O rows, copies of the psum
                denominator row, one wide fast reciprocal, bf16 cast of it,
                2 multiplies.  GpSimd: 2 bf16 partition broadcasts (its
                only op type, so no Q7 library thrash).  For the last
                stripe the denominator copies run on the (now idle) scalar
                engine so the chain to outproj(3) is shorter.
                """
                blk = ((hp * 4 + tta) * 2) * TA
                if last:
                    nc.scalar.activation(
                        rec_all[0:1, blk + TA : blk + 2 * TA],
                        ps_o[64:65, TA : 2 * TA],
                        AF.Copy,
                    )
                    nc.vector.tensor_copy(
                        ot_sb[hp][64:128, bass.ts(tta, TA)],
                        ps_o[0:64, TA : 2 * TA],
                    )
                    norm_head(hp, tta, 1, blk)
                    return
                nc.vector.tensor_copy(
                    rec_all[0:1, blk + TA : blk + 2 * TA],
                    ps_o[64:65, TA : 2 * TA],
                )
                nc.vector.tensor_copy(
                    ot_sb[hp][64:128, bass.ts(tta, TA)],
                    ps_o[0:64, TA : 2 * TA],
                )
                nc.vector.reciprocal_approx_fast(
                    out=rec_all[0:1, blk : blk + 2 * TA],
                    in_=rec_all[0:1, blk : blk + 2 * TA],
                )
                nc.vector.tensor_copy(
                    den_bf[0:1, blk : blk + 2 * TA],
                    rec_all[0:1, blk : blk + 2 * TA],
                )
                for h in range(2):
                    r_sb = misc.tile(
                        [128, TA], DT, tag="rsb", bufs=4, name="r_sb"
                    )
                    nc.gpsimd.partition_broadcast(
                        r_sb[:], den_bf[0:1, blk + h * TA : blk + h * TA + TA]
                    )
                    hb = 64 * h
                    nc.vector.tensor_mul(
                        ot_sb[hp][hb : hb + 64, bass.ts(tta, TA)],
                        ot_sb[hp][hb : hb + 64, bass.ts(tta, TA)],
                        r_sb[hb : hb + 64, :],
                    )

            def emit_stripes(filler_pushes):
                """All 8 stripes as one pipelined stream: scores+exp run two
                s-chunks ahead of PV and cross stripe boundaries, so the ACT
                exp stream never waits on a ps_o drain.  filler_pushes maps
                stripe index -> (thunk list, start_sc, rate); thunks are
                popped inside that stripe at `rate` per iteration once
                sc >= start_sc."""
                from collections import deque

                fillers = deque()
                NS = len(STRIPES)
                pts = {}
                DEPTH = 3

                def prefetch(gi):
                    if gi < NS * N_SC:
                        pts[gi] = emit_scores(gi // N_SC, gi % N_SC)

                for gi in range(DEPTH):
                    prefetch(gi)
                for si in range(NS):
                    hp, tta = STRIPES[si]
                    extra = filler_pushes.get(si)
                    if extra:
                        fillers.extend(extra[0])
                        start_sc, rate = extra[1], extra[2]
                    else:
                        start_sc, rate = 0, 2.5
                    popped = 0
                    ps_o = psacc.tile([65, 1024], F32, tag="acc", name="ps_o")
                    for sc in range(N_SC):
                        # pop fillers FIRST so anything a prefetched score
                        # or PV depends on is already in the stream
                        if sc >= start_sc:
                            want = int((sc - start_sc + 1) * rate)
                            while popped < want and fillers:
                                fillers.popleft()()
                                popped += 1
                        pt = pts.pop(si * N_SC + sc)
                        for h in range(2):
                            nc.tensor.matmul(
                                ps_o[:, bass.ts(h, TA)],
                                vn_sb[hp][:, sc, h, 0:65],
                                pt[:, bass.ts(h, TA)],
                                start=(sc == 0),
                                stop=(sc == N_SC - 1),
                            )
                            if sc == N_SC - 1 and h == 0:
                                emit_tail_h0(
                                    hp, tta, ps_o, last=(si == NS - 1)
                                )
                        prefetch(si * N_SC + sc + DEPTH)
                    emit_tail(hp, tta, ps_o, last=(si == NS - 1))
                    # keep the PE warm across the boundary: a couple of
                    # filler thunks sit between this stripe's last PV and
                    # the next stripe's first (drain-gated) PV
                    for _ in range(2):
                        if fillers:
                            fillers.popleft()()
                            popped += 1
                while fillers:
                    fillers.popleft()()

            def emit_outproj_thunks(tta, store_on_act=False):
                """Output projection for one 512-query stripe: 4 t-chunks,
                each = 2 psum-half fills + 1 store thunk."""
                thunks = []
                for j in range(TA // 128):
                    tc_ = tta * (TA // 128) + j
                    tsl = bass.ts(tc_, 128)
                    state = {}
                    for half in range(2):
                        def mmf(half=half, state=state, tsl=tsl, j=j):
                            # final chunks alternate psum pools (scores are
                            # done, psmm is free) so chunk j+1's matmuls
                            # don't wait on chunk j's store copies
                            if store_on_act and j % 2:
                                ps = psmm.tile(
                                    [128, 512], F32, tag="mm", name="ps_out"
                                )
                            else:
                                ps = psout.tile(
                                    [128, 512], F32, tag="out", name="ps_out"
                                )
                            state[half] = ps
                            hsl = bass.ts(half, 512)
                            for hp in range(HP):
                                nc.tensor.matmul(
                                    ps[:],
                                    ot_sb[hp][:, tsl],
                                    wot_sb[:, hp, hsl],
                                    start=(hp == 0),
                                    stop=(hp == HP - 1),
                                )
                        thunks.append(mmf)

                    def store(state=state, tc_=tc_):
                        o_t = misc.tile(
                            [128, 1024], DT, tag="out", bufs=4, name="o_t"
                        )
                        for half in range(2):
                            # final stores split halves across ACT and DVE
                            # so the two copies run in parallel
                            if store_on_act and half == 0:
                                nc.scalar.activation(
                                    o_t[:, bass.ts(half, 512)],
                                    state[half][:],
                                    AF.Copy,
                                )
                            else:
                                nc.vector.tensor_copy(
                                    o_t[:, bass.ts(half, 512)], state[half][:]
                                )
                        # split the store so the o_t slot frees fast; the
                        # final stores alternate issue engines so the sync
                        # sequencer's ~0.6us/dma doesn't serialize the tail
                        for g in range(4):
                            r0 = tc_ * 128 + g * 32
                            eng = (
                                dma_a if store_on_act and g % 2 else dma_s
                            )
                            eng(
                                out[r0 : r0 + 32, :],
                                o_t[bass.ts(g, 32), :],
                            )
                    thunks.append(store)
                return thunks

            # ---- emission schedule ---------------------------------------
            # Prefix: the minimum hp0 projection work to start stripe
            # (0, 0) — K/V/ones for t-tile 0 plus the first Q half.
            # Everything else — K/V t-tile 1, remaining Q, all hp1
            # projections, and the output projections — runs as paced
            # fillers inside the stripes.  outproj(tta) is hosted in
            # stripe (hp1, tta+1); outproj(3) trails.
            # one-time broadcast of the V bias row (also warms the GpSimd
            # broadcast library well before the first stripe tail)
            nc.gpsimd.partition_broadcast(bvbc[:], bvt_sb[:])

            emit_vn_ones(0)
            emit_vn_ones(1)
            # Prefix: K t-tile 0 for BOTH head pairs (interleaved so the
            # DMA-paced stretch feeds double matmul work), V^T chunks 0-3
            # and the first Q half for both pairs.
            emit_proj_pair("ps_k", wk_sb, bk_sb, xfr_sb, kt_sb, 0)
            for sc in range(4):
                for hp in range(HP):
                    for f in vt_thunks(hp, sc):
                        f()
            for hp in range(HP):
                t, _ = proj_half_thunks(
                    "ps_q", wq_sb, bq_sb, xto_sb, qt_sb, hp, 0, 0
                )
                for f in t:
                    f()

            proj_rest = []

            def add_half(ps_name, w_sb, b_sb, x_sb, dst, hp, tt, half):
                t, _ = proj_half_thunks(
                    ps_name, w_sb, b_sb, x_sb, dst, hp, tt, half
                )
                proj_rest.extend(t)

            # ordered by the stripe-0 deadlines at rate 3.5 with
            # pops-before-PV (see pacing analysis in session notes)
            for sc in range(4, 8):
                proj_rest += vt_thunks(0, sc)
            add_half("ps_k", wk_sb, bk_sb, xfr_sb, kt_sb, 0, 1, 0)
            for sc in range(8, 12):
                proj_rest += vt_thunks(0, sc)
            add_half("ps_k", wk_sb, bk_sb, xfr_sb, kt_sb, 0, 1, 1)
            for sc in range(12, 14):
                proj_rest += vt_thunks(0, sc)
            add_half("ps_q", wq_sb, bq_sb, xto_sb, qt_sb, 0, 0, 1)
            for sc in range(14, 16):
                proj_rest += vt_thunks(0, sc)
            add_half("ps_q", wq_sb, bq_sb, xto_sb, qt_sb, 0, 1, 0)
            add_half("ps_q", wq_sb, bq_sb, xto_sb, qt_sb, 0, 1, 1)
            # remaining hp1 work (K tt0 / V^T 0-3 / Q h0 ran in the prefix)
            for tt_half in ((1, 0), (1, 1)):
                add_half(
                    "ps_k", wk_sb, bk_sb, xfr_sb, kt_sb, 1, tt_half[0],
                    tt_half[1],
                )
            for sc in range(4, 16):
                proj_rest += vt_thunks(1, sc)
            add_half("ps_q", wq_sb, bq_sb, xto_sb, qt_sb, 1, 0, 1)
            add_half("ps_q", wq_sb, bq_sb, xto_sb, qt_sb, 1, 1, 0)
            # Q(1,1) h1 is held out of proj_rest: its halves ride with the
            # outproj pushes in stripes 5/6 (their deadlines are that late)
            # so those stripes' boundary pops still find ready PE work.
            q11h1, _ = proj_half_thunks(
                "ps_q", wq_sb, bq_sb, xto_sb, qt_sb, 1, 1, 1
            )

            filler_pushes = {0: (proj_rest, 0, 3.5)}
            filler_pushes[5] = (
                q11h1 + emit_outproj_thunks(0), 5, 1.4,
            )
            filler_pushes[6] = (emit_outproj_thunks(1), 5, 1.0)
            filler_pushes[7] = (emit_outproj_thunks(2), 5, 1.0)
            emit_stripes(filler_pushes)
            for f in emit_outproj_thunks(3, store_on_act=True):
                f()

    nc.compile()
    return nc


def _prep_in_maps(x_to, x_from, Wq, bq, Wk, bk, Wv, bv, Wo):
    scale = 1.0 / np.sqrt(np.float32(DH))
    # [H, D, DH] -> [D, H*DH] with column h*DH+d
    wq_f = np.ascontiguousarray(Wq.transpose(1, 0, 2).reshape(D, H * DH)) * scale
    wk_f = np.ascontiguousarray(Wk.transpose(1, 0, 2).reshape(D, H * DH))
    wv_f = np.ascontiguousarray(Wv.transpose(1, 0, 2).reshape(D, H * DH))
    bq_f = bq.reshape(H * DH) * scale
    bk_f = bk.reshape(H * DH)
    bv_f = bv.reshape(H * DH)

    xt_to = np.ascontiguousarray(x_to.transpose(0, 2, 1))    # [B, D, TQ]
    xt_from = np.ascontiguousarray(x_from.transpose(0, 2, 1))

    def f32(a):
        return np.ascontiguousarray(a, dtype=np.float32)

    import ml_dtypes

    def fdt(a):
        return np.ascontiguousarray(a, dtype=ml_dtypes.bfloat16)

    in_maps = []
    for c in range(N_CORES):
        b, g = divmod(c, HEADS_PER_CORE)
        cs = slice(g * 256, (g + 1) * 256)
        in_maps.append(
            {
                "xt_to": fdt(xt_to[b]),
                "xt_from": fdt(xt_from[b]),
                "wq": fdt(wq_f[:, cs]),
                "wk": fdt(wk_f[:, cs]),
                "wv": fdt(wv_f[:, cs]),
                # [256] -> [2 pairs, 128] -> [128, 2]
                "bq": f32(bq_f[cs].reshape(2, 128).T),
                "bk": f32(bk_f[cs].reshape(2, 128).T),
                "bvt": fdt(bv_f[cs].reshape(1, 256)),
                # Wo[:, cs].T = [256, 1024] -> [2, 128, 1024] -> [128, 2, 1024]
                "wot": fdt(
                    np.ascontiguousarray(Wo[:, cs].T)
                    .reshape(2, 128, 1024)
                    .transpose(1, 0, 2)
                ),
            }
        )
    return in_maps


LAST_EXEC_TIME_NS = None
LAST_TRACE = None


def kernel(x_to, x_from, Wq, bq, Wk, bk, Wv, bv, Wo, bo):
    global LAST_EXEC_TIME_NS, LAST_TRACE
    if "nc" not in _CACHED:
        _CACHED["nc"] = build_program()
    nc = _CACHED["nc"]

    in_maps = _prep_in_maps(
        np.asarray(x_to), np.asarray(x_from), np.asarray(Wq), np.asarray(bq),
        np.asarray(Wk), np.asarray(bk), np.asarray(Wv), np.asarray(bv),
        np.asarray(Wo),
    )
    res = run_bass_kernel_spmd(nc, in_maps, list(range(N_CORES)))
    LAST_EXEC_TIME_NS = res.exec_time_ns
    LAST_TRACE = res.instructions_and_trace

    out = np.zeros((B, TQ, D), dtype=np.float32)
    for c in range(N_CORES):
        out[c // HEADS_PER_CORE] += res.results[c]["out"].astype(np.float32)
    out += np.asarray(bo, dtype=np.float32)
    return out


# revision 85
# speedup vs baseline: 1.0216x; 1.0146x over previous
"""Trainium2 Bass kernel for 16-head MultiHeadAttention (B=2, T=2048, D=1024).

Sharding (8 NeuronCores): core c handles batch b = c//4 and head group
g = c%4 (heads 4g..4g+3).  Each core computes Q/K/V projections for its 4
heads, attention, and a partial output projection against its 256 rows of
W_O.  The host sums the 4 partials per batch and adds b_O (row-parallel TP;
the all-reduce is folded into the unshard step).

Device layout notes:
 - The host pre-transposes x to x^T [D, T] so the contraction dim (features)
   lands on SBUF partitions without any on-device transposes of x.
 - Attention is computed in the S^T = K @ Q^T orientation: the softmax
   denominator is then a partition-axis sum, which the PE produces for free
   via a ones-column appended to V (out = [V|1]^T @ P^T gives O^T rows 0..63
   and the denominator in row 64).
 - Per head pair (2 heads of 64), weights are stacked to fill 128 partitions.
 - Matmul operands are bf16 (fp32 PSUM accumulation).

Schedule notes (final):
 - V^T is computed directly on the PE (stationary = x^T key-chunk,
   moving = Wv) so no transposes / vn copies exist; bias lands via one
   strided DVE add against a broadcast bias tile.
 - All 8 attention stripes run as ONE software-pipelined stream: scores
   for global chunk g+3 are emitted (and exp'd) before the PV matmuls of
   chunk g, so the ACT exp stream crosses stripe boundaries without
   waiting on the ps_o drain.
 - Only a minimal prefix (K t-tile 0 and V^T/Q first halves for BOTH
   head pairs, interleaved per f-chunk to ride the DMA-paced start) runs
   before the stripes; every other projection and the output projection
   are paced filler thunks popped inside the stripes, sized against
   emission-order deadlines (a filler a prefetched score depends on must
   be emitted before it, or the in-order PE queue deadlocks).
 - Stripe tails: drain ps_o with 2 bf16 casts + denominator-row copies
   (split around the last PV), one wide reciprocal_approx_fast, bf16
   cast, GpSimd partition broadcasts, in-place multiplies.  The last
   stripe runs per-head chains with the copies on the idle ACT engine
   so the trailing outproj(3) starts ~3us sooner.
 - outproj psum lives on its own 1-bank pool (2 bufs); the final chunks
   alternate onto the freed score slots so matmuls don't serialize on
   store copies.  Output partials are stored bf16 (host sums in f32).
 - Input DMAs are sliced [128, 512-col] and issued on two sequencers
   (sync + scalar) in consumption order; output stores are split 4-way
   so SBUF staging slots free quickly.
"""

import os
import sys

import numpy as np

for _p in ("/opt/trn_rl_repo", "/root/.axon_site/_ro/trn_rl_repo"):
    if os.path.isdir(_p) and _p not in sys.path:
        sys.path.insert(0, _p)

import concourse.bass as bass
import concourse.mybir as mybir
import concourse.tile as tile
from concourse import bacc
from concourse.bass_utils import run_bass_kernel_spmd

F32 = mybir.dt.float32
BF16 = mybir.dt.bfloat16
AF = mybir.ActivationFunctionType

B, TQ, TK = 2, 2048, 2048
D = 1024          # model dim == x_to/x_from feature dim
H, DH = 16, 64
N_CORES = 8
HEADS_PER_CORE = 4   # one batch per core
HP = 2               # head pairs per core (2 heads of 64 stacked -> 128)

TT = 1024            # t-tile (bf16 moving free dim max)
N_TT = TQ // TT      # 2
N_SC = TK // 128     # 16 s-chunks
N_FC = D // 128      # 8 f-chunks
TA = 512             # attention stripe width (psum bank in f32)

DT = BF16

_CACHED = {}


def build_program():
    from collections import deque

    nc = bacc.Bacc(
        "TRN2", target_bir_lowering=False, debug=False, num_devices=N_CORES
    )

    xt_to = nc.dram_tensor("xt_to", [D, TQ], DT, kind="ExternalInput")
    xt_from = nc.dram_tensor("xt_from", [D, TK], DT, kind="ExternalInput")
    wq = nc.dram_tensor("wq", [D, 256], DT, kind="ExternalInput")
    wk = nc.dram_tensor("wk", [D, 256], DT, kind="ExternalInput")
    wv = nc.dram_tensor("wv", [D, 256], DT, kind="ExternalInput")
    bq = nc.dram_tensor("bq", [128, 2], F32, kind="ExternalInput")
    bk = nc.dram_tensor("bk", [128, 2], F32, kind="ExternalInput")
    bvt = nc.dram_tensor("bvt", [1, 256], DT, kind="ExternalInput")
    wot = nc.dram_tensor("wot", [128, 2, 1024], DT, kind="ExternalInput")
    # bf16 partials: host sums 4 per batch in f32; rounding ~0.2% << gate
    out = nc.dram_tensor("out", [TQ, D], DT, kind="ExternalOutput")

    with tile.TileContext(nc) as tc:
        with (
            tc.tile_pool(name="wpool", bufs=1) as wpool,
            tc.tile_pool(name="actpool", bufs=1) as actpool,
            tc.tile_pool(name="ptpool", bufs=4) as ptpool,
            tc.tile_pool(name="misc", bufs=2) as misc,
            tc.tile_pool(name="psmm", bufs=2, space="PSUM") as psmm,
            tc.tile_pool(name="psacc", bufs=1, space="PSUM") as psacc,
            tc.tile_pool(name="psout", bufs=2, space="PSUM") as psout,
        ):
            # ---- constants & weights -------------------------------------
            wq_sb = wpool.tile([128, N_FC, 256], DT)
            wk_sb = wpool.tile([128, N_FC, 256], DT)
            wv_sb = wpool.tile([128, N_FC, 256], DT)
            bq_sb = wpool.tile([128, 2], F32)
            bk_sb = wpool.tile([128, 2], F32)
            bvt_sb = wpool.tile([1, 256], DT)
            # bv broadcast across rows, indexed [hp, h, d]
            bvbc = wpool.tile([128, HP, 2, 64], DT)
            wot_sb = wpool.tile([128, 2, 1024], DT)

            # PE warmup: throwaway matmuls on a zeroed tile so the tensor
            # engine's p-state ramps to full clock while the first x
            # slices are still in flight; the real prefix then runs at
            # ~216ns/512cols instead of the cold 427-585ns.
            warm = wpool.tile([128, 512], DT, name="warm")
            nc.vector.memset(warm[:], 0.0)

            def emit_warm():
                ps_w = psout.tile([128, 512], F32, tag="out", name="ps_w")
                nc.tensor.matmul(
                    ps_w[:], warm[:, 0:128], warm[:], start=True, stop=True
                )

            # bridge from kernel start (~8us) to first x-slice arrival
            # (~14.8us): any idle gap resets the ramp, so keep going
            for _w in range(24):
                emit_warm()

            # x^T resident in SBUF
            xfr_sb = actpool.tile([128, N_FC, TK], DT, name="xfr_sb")
            xto_sb = actpool.tile([128, N_FC, TQ], DT, name="xto_sb")
            xt_to_r = xt_to.rearrange("(c p) t -> p c t", p=128)
            xt_from_r = xt_from.rearrange("(c p) t -> p c t", p=128)
            wq_r = wq.rearrange("(c p) d -> p c d", p=128)
            wk_r = wk.rearrange("(c p) d -> p c d", p=128)
            wv_r = wv.rearrange("(c p) d -> p c d", p=128)

            # ---- input DMAs, sliced + ordered by first consumption -------
            # Three issue streams in parallel (~0.6us sequencer cost per
            # dma each): sync carries x_from, vector carries weights + x_to,
            # scalar (short queue — it has early transpose copies) carries
            # wq/wot.  Slicing spreads each tensor across many of the 16
            # DMA engines so the first consumers start a few us in.
            dma_s = nc.sync.dma_start
            dma_a = nc.scalar.dma_start
            for fcp in range(2):            # K weights first
                dma_a(
                    wk_sb[:, 4 * fcp : 4 * fcp + 4, :],
                    wk_r[:, 4 * fcp : 4 * fcp + 4, :],
                )
            # head-critical x slices alternate across both issue queues so
            # their ~0.6us/dma sequencer costs run in parallel: the prefix
            # consumes x_from cols 0:1024 then x_to cols 0:512.
            def dma_x(i, dst, src):
                (dma_s if i % 2 else dma_a)(dst, src)

            i = 0
            for fc in range(N_FC):          # x_from cols 0:512 (K h0, V^T)
                dma_x(i, xfr_sb[:, fc, 0:512], xt_from_r[:, fc, 0:512])
                i += 1
            dma_a(bk_sb[:], bk[:])
            dma_a(bvt_sb[:], bvt[:])
            dma_a(bq_sb[:], bq[:])
            for fc in range(N_FC):          # x_from cols 512:1024 (K h1)
                dma_x(i, xfr_sb[:, fc, 512:1024], xt_from_r[:, fc, 512:1024])
                i += 1
            for fcp in range(2):            # V weights
                dma_a(
                    wv_sb[:, 4 * fcp : 4 * fcp + 4, :],
                    wv_r[:, 4 * fcp : 4 * fcp + 4, :],
                )
            for fcp in range(2):            # Q weights
                dma_a(
                    wq_sb[:, 4 * fcp : 4 * fcp + 4, :],
                    wq_r[:, 4 * fcp : 4 * fcp + 4, :],
                )
            for fc in range(N_FC):          # x_to cols 0:512 (Q prefix half)
                dma_x(i, xto_sb[:, fc, 0:512], xt_to_r[:, fc, 0:512])
                i += 1
            # the rest in consumption order on sync
            for cs in range(2, 4):          # x_from cols 1024:2048 (tt1)
                sl = slice(cs * 512, cs * 512 + 512)
                for fc in range(N_FC):
                    dma_s(xfr_sb[:, fc, sl], xt_from_r[:, fc, sl])
            for cs in range(1, 4):          # x_to cols 512:2048
                sl = slice(cs * 512, cs * 512 + 512)
                for fc in range(N_FC):
                    dma_s(xto_sb[:, fc, sl], xt_to_r[:, fc, sl])
            for hp in range(HP):            # output proj weights (used late)
                dma_a(wot_sb[:, hp, :], wot[:, hp, :])

            # ---- persistent activations ----------------------------------
            qt_sb = [
                actpool.tile([128, TQ], DT, name=f"qt{hp}") for hp in range(HP)
            ]
            kt_sb = [
                actpool.tile([128, TK], DT, name=f"kt{hp}") for hp in range(HP)
            ]
            # V^T per s-chunk: [keys, head, dim | ones col]
            vn_sb = [
                actpool.tile([128, N_SC, 2, 65], DT, name=f"vn{hp}")
                for hp in range(HP)
            ]
            ot_sb = [
                actpool.tile([128, TQ], DT, name=f"ot{hp}") for hp in range(HP)
            ]
            # softmax denominator reciprocals on partition 0: (hp, h) at
            # offset (2*hp+h)*TQ; filled by reciprocal_approx_fast straight
            # from the PSUM denominator row.
            rec_all = actpool.tile([1, 4 * TQ], F32, name="rec_all")
            den_bf = actpool.tile([1, 4 * TQ], DT, name="den_bf")

            def emit_proj_pair(ps_name, w_sb, b_sb, x_sb, dst, tt):
                """Direct (prefix) projection tile for BOTH head pairs on
                the two psmm slots, interleaved per f-chunk so each
                arriving x slice feeds twice the matmul work while the
                prefix is DMA-paced."""
                ps = [
                    psmm.tile([128, TT], F32, tag="mm", name=f"{ps_name}{hp}")
                    for hp in range(HP)
                ]
                for half in range(2):
                    for fc in range(N_FC):
                        for hp in range(HP):
                            nc.tensor.matmul(
                                ps[hp][:, bass.ts(half, 512)],
                                w_sb[:, fc, bass.ts(hp, 128)],
                                x_sb[
                                    :, fc,
                                    tt * TT + half * 512 : tt * TT
                                    + half * 512
                                    + 512,
                                ],
                                start=(fc == 0),
                                stop=(fc == N_FC - 1),
                            )
                        # a warm matmul bridges the x-slice stagger so the
                        # DMA-paced fill can't reset the PE clock ramp
                        if fc < N_FC - 1:
                            emit_warm()
                for hp in range(HP):
                    nc.vector.tensor_scalar_add(
                        dst[hp][:, bass.ts(tt, TT)],
                        ps[hp][:],
                        b_sb[:, hp : hp + 1],
                    )

            def emit_vn_ones(hp):
                nc.vector.memset(vn_sb[hp][:, :, :, 64], 1.0)

            def vt_thunks(hp, sc):
                """V^T for one 128-key chunk, computed directly on the PE:
                stationary = x^T chunk [feat, keys], moving = Wv slice
                [feat, 2*64 dims] -> psum [keys, 2, 64]; bias lands via one
                strided DVE add against the broadcast bias tile."""
                state = {}

                def mk_mm(g):
                    def mm(g=g):
                        if g == 0:
                            state["ps"] = psout.tile(
                                [128, 2, 64], F32, tag="out", name="ps_vt"
                            )
                        for fc in range(4 * g, 4 * g + 4):
                            nc.tensor.matmul(
                                state["ps"][:],
                                xfr_sb[:, fc, sc * 128 : sc * 128 + 128],
                                wv_sb[:, fc, bass.ts(hp, 128)],
                                start=(fc == 0),
                                stop=(fc == N_FC - 1),
                            )
                    return mm

                def add():
                    nc.vector.tensor_add(
                        vn_sb[hp][:, sc, :, 0:64],
                        state["ps"][:],
                        bvbc[:, hp, :, :],
                    )
                return [mk_mm(0), mk_mm(1), add]

            def proj_half_thunks(ps_name, w_sb, b_sb, x_sb, dst, hp, tt, half):
                """Filler-style K/Q projection: one 512-col half on a
                1-bank psout tile; 4 matmul thunks (2 f-chunks each) +
                copyback."""
                dsl = bass.ts(hp, 128)
                col0 = tt * TT + half * 512
                state = {}

                def mk_mm(fcp):
                    def mm(fcp=fcp):
                        if fcp == 0:
                            state["ps"] = psout.tile(
                                [128, 512], F32, tag="out", name=ps_name
                            )
                        for fc in (2 * fcp, 2 * fcp + 1):
                            nc.tensor.matmul(
                                state["ps"][:],
                                w_sb[:, fc, dsl],
                                x_sb[:, fc, col0 : col0 + 512],
                                start=(fc == 0),
                                stop=(fc == N_FC - 1),
                            )
                    return mm

                thunks = [mk_mm(fcp) for fcp in range(4)]

                def copyback():
                    nc.vector.tensor_scalar_add(
                        dst[hp][:, col0 : col0 + 512],
                        state["ps"][:],
                        b_sb[:, hp : hp + 1],
                    )
                thunks.append(copyback)
                return thunks, copyback

            STRIPES = [(hp, tta) for hp in range(HP) for tta in range(TQ // TA)]

            def emit_scores(si, sc):
                hp, tta = STRIPES[si]
                ps_s = psmm.tile([128, 1024], F32, tag="mm", name="ps_s")
                for h in range(2):
                    hb = 64 * h
                    nc.tensor.matmul(
                        ps_s[:, bass.ts(h, TA)],
                        kt_sb[hp][hb : hb + 64, bass.ts(sc, 128)],
                        qt_sb[hp][hb : hb + 64, bass.ts(tta, TA)],
                        start=True,
                        stop=True,
                    )
                pt = ptpool.tile([128, 1024], DT, tag="pt", name="pt")
                nc.scalar.activation(pt[:], ps_s[:], AF.Exp)
                return pt

            def norm_head(hp, tta, h, blk):
                """Reciprocal -> bf16 cast (ACT) -> broadcast -> multiply
                for one head; used for the last stripe where chain latency
                gates the trailing output projection."""
                o = blk + h * TA
                nc.vector.reciprocal_approx_fast(
                    out=rec_all[0:1, o : o + TA],
                    in_=rec_all[0:1, o : o + TA],
                )
                nc.scalar.activation(
                    den_bf[0:1, o : o + TA], rec_all[0:1, o : o + TA], AF.Copy
                )
                r_sb = misc.tile([128, TA], DT, tag="rsb", bufs=4, name="r_sb")
                nc.gpsimd.partition_broadcast(r_sb[:], den_bf[0:1, o : o + TA])
                hb = 64 * h
                nc.vector.tensor_mul(
                    ot_sb[hp][hb : hb + 64, bass.ts(tta, TA)],
                    ot_sb[hp][hb : hb + 64, bass.ts(tta, TA)],
                    r_sb[hb : hb + 64, :],
                )

            def emit_tail_h0(hp, tta, ps_o, last=False):
                """First half of the ps_o drain — emitted right after
                PV(15, h0) so it overlaps PV(15, h1).  For the last stripe
                the whole h0 normalize chain starts here too."""
                blk = ((hp * 4 + tta) * 2) * TA
                nc.vector.tensor_copy(
                    rec_all[0:1, blk : blk + TA], ps_o[64:65, 0:TA]
                )
                nc.vector.tensor_copy(
                    ot_sb[hp][0:64, bass.ts(tta, TA)], ps_o[0:64, 0:TA]
                )
                if last:
                    norm_head(hp, tta, 0, blk)

            def emit_tail(hp, tta, ps_o, last=False):
                """Rest of the drain + normalize ot in place.

                DVE: bf16 casts of the O rows, copies of the psum
                denominator row, one wide fast reciprocal, bf16 cast of it,
                2 multiplies.  GpSimd: 2 bf16 partition broadcasts (its
                only op type, so no Q7 library thrash).  For the last
                stripe the denominator copies run on the (now idle) scalar
                engine so the chain to outproj(3) is shorter.
                """
                blk = ((hp * 4 + tta) * 2) * TA
                if last:
                    nc.scalar.activation(
                        rec_all[0:1, blk + TA : blk + 2 * TA],
                        ps_o[64:65, TA : 2 * TA],
                        AF.Copy,
                    )
                    nc.vector.tensor_copy(
                        ot_sb[hp][64:128, bass.ts(tta, TA)],
                        ps_o[0:64, TA : 2 * TA],
                    )
                    norm_head(hp, tta, 1, blk)
                    return
                nc.vector.tensor_copy(
                    rec_all[0:1, blk + TA : blk + 2 * TA],
                    ps_o[64:65, TA : 2 * TA],
                )
                nc.vector.tensor_copy(
                    ot_sb[hp][64:128, bass.ts(tta, TA)],
                    ps_o[0:64, TA : 2 * TA],
                )
                nc.vector.reciprocal_approx_fast(
                    out=rec_all[0:1, blk : blk + 2 * TA],
                    in_=rec_all[0:1, blk : blk + 2 * TA],
                )
                nc.vector.tensor_copy(
                    den_bf[0:1, blk : blk + 2 * TA],
                    rec_all[0:1, blk : blk + 2 * TA],
                )
                for h in range(2):
                    r_sb = misc.tile(
                        [128, TA], DT, tag="rsb", bufs=4, name="r_sb"
                    )
                    nc.gpsimd.partition_broadcast(
                        r_sb[:], den_bf[0:1, blk + h * TA : blk + h * TA + TA]
                    )
                    hb = 64 * h
                    nc.vector.tensor_mul(
                        ot_sb[hp][hb : hb + 64, bass.ts(tta, TA)],
                        ot_sb[hp][hb : hb + 64, bass.ts(tta, TA)],
                        r_sb[hb : hb + 64, :],
                    )

            def emit_stripes(filler_pushes):
                """All 8 stripes as one pipelined stream: scores+exp run two
                s-chunks ahead of PV and cross stripe boundaries, so the ACT
                exp stream never waits on a ps_o drain.  filler_pushes maps
                stripe index -> (thunk list, start_sc, rate); thunks are
                popped inside that stripe at `rate` per iteration once
                sc >= start_sc."""
                from collections import deque

                fillers = deque()
                NS = len(STRIPES)
                pts = {}
                DEPTH = 3

                def prefetch(gi):
                    if gi < NS * N_SC:
                        pts[gi] = emit_scores(gi // N_SC, gi % N_SC)

                for gi in range(DEPTH):
                    prefetch(gi)
                for si in range(NS):
                    hp, tta = STRIPES[si]
                    extra = filler_pushes.get(si)
                    if extra:
                        fillers.extend(extra[0])
                        start_sc, rate = extra[1], extra[2]
                    else:
                        start_sc, rate = 0, 2.5
                    popped = 0
                    ps_o = psacc.tile([65, 1024], F32, tag="acc", name="ps_o")
                    for sc in range(N_SC):
                        # pop fillers FIRST so anything a prefetched score
                        # or PV depends on is already in the stream
                        if sc >= start_sc:
                            want = int((sc - start_sc + 1) * rate)
                            while popped < want and fillers:
                                fillers.popleft()()
                                popped += 1
                        pt = pts.pop(si * N_SC + sc)
                        for h in range(2):
                            nc.tensor.matmul(
                                ps_o[:, bass.ts(h, TA)],
                                vn_sb[hp][:, sc, h, 0:65],
                                pt[:, bass.ts(h, TA)],
                                start=(sc == 0),
                                stop=(sc == N_SC - 1),
                            )
                            if sc == N_SC - 1 and h == 0:
                                emit_tail_h0(
                                    hp, tta, ps_o, last=(si == NS - 1)
                                )
                        prefetch(si * N_SC + sc + DEPTH)
                    emit_tail(hp, tta, ps_o, last=(si == NS - 1))
                    # keep the PE warm across the boundary: a couple of
                    # filler thunks sit between this stripe's last PV and
                    # the next stripe's first (drain-gated) PV
                    for _ in range(2):
                        if fillers:
                            fillers.popleft()()
                            popped += 1
                while fillers:
                    fillers.popleft()()

            def emit_outproj_thunks(tta, store_on_act=False):
                """Output projection for one 512-query stripe: 4 t-chunks,
                each = 2 psum-half fills + 1 store thunk."""
                thunks = []
                for j in range(TA // 128):
                    tc_ = tta * (TA // 128) + j
                    tsl = bass.ts(tc_, 128)
                    state = {}
                    for half in range(2):
                        def mmf(half=half, state=state, tsl=tsl, j=j):
                            # final chunks alternate psum pools (scores are
                            # done, psmm is free) so chunk j+1's matmuls
                            # don't wait on chunk j's store copies
                            if store_on_act and j % 2:
                                ps = psmm.tile(
                                    [128, 512], F32, tag="mm", name="ps_out"
                                )
                            else:
                                ps = psout.tile(
                                    [128, 512], F32, tag="out", name="ps_out"
                                )
                            state[half] = ps
                            hsl = bass.ts(half, 512)
                            for hp in range(HP):
                                nc.tensor.matmul(
                                    ps[:],
                                    ot_sb[hp][:, tsl],
                                    wot_sb[:, hp, hsl],
                                    start=(hp == 0),
                                    stop=(hp == HP - 1),
                                )
                        thunks.append(mmf)

                    def store(state=state, tc_=tc_):
                        o_t = misc.tile(
                            [128, 1024], DT, tag="out", bufs=4, name="o_t"
                        )
                        for half in range(2):
                            # final stores split halves across ACT and DVE
                            # so the two copies run in parallel
                            if store_on_act and half == 0:
                                nc.scalar.activation(
                                    o_t[:, bass.ts(half, 512)],
                                    state[half][:],
                                    AF.Copy,
                                )
                            else:
                                nc.vector.tensor_copy(
                                    o_t[:, bass.ts(half, 512)], state[half][:]
                                )
                        # split the store so the o_t slot frees fast; the
                        # final stores alternate issue engines so the sync
                        # sequencer's ~0.6us/dma doesn't serialize the tail
                        for g in range(4):
                            r0 = tc_ * 128 + g * 32
                            eng = (
                                dma_a if store_on_act and g % 2 else dma_s
                            )
                            eng(
                                out[r0 : r0 + 32, :],
                                o_t[bass.ts(g, 32), :],
                            )
                    thunks.append(store)
                return thunks

            # ---- emission schedule ---------------------------------------
            # Prefix: the minimum hp0 projection work to start stripe
            # (0, 0) — K/V/ones for t-tile 0 plus the first Q half.
            # Everything else — K/V t-tile 1, remaining Q, all hp1
            # projections, and the output projections — runs as paced
            # fillers inside the stripes.  outproj(tta) is hosted in
            # stripe (hp1, tta+1); outproj(3) trails.
            # one-time broadcast of the V bias row (also warms the GpSimd
            # broadcast library well before the first stripe tail)
            nc.gpsimd.partition_broadcast(bvbc[:], bvt_sb[:])

            emit_vn_ones(0)
            emit_vn_ones(1)
            # Prefix: K t-tile 0 for BOTH head pairs (interleaved so the
            # DMA-paced stretch feeds double matmul work), V^T chunks 0-3
            # and the first Q half for both pairs.
            emit_proj_pair("ps_k", wk_sb, bk_sb, xfr_sb, kt_sb, 0)
            for sc in range(4):
                for hp in range(HP):
                    for f in vt_thunks(hp, sc):
                        f()
            for hp in range(HP):
                t, _ = proj_half_thunks(
                    "ps_q", wq_sb, bq_sb, xto_sb, qt_sb, hp, 0, 0
                )
                for f in t:
                    f()

            proj_rest = []

            def add_half(ps_name, w_sb, b_sb, x_sb, dst, hp, tt, half):
                t, _ = proj_half_thunks(
                    ps_name, w_sb, b_sb, x_sb, dst, hp, tt, half
                )
                proj_rest.extend(t)

            # ordered by the stripe-0 deadlines at rate 3.5 with
            # pops-before-PV (see pacing analysis in session notes)
            for sc in range(4, 8):
                proj_rest += vt_thunks(0, sc)
            add_half("ps_k", wk_sb, bk_sb, xfr_sb, kt_sb, 0, 1, 0)
            for sc in range(8, 12):
                proj_rest += vt_thunks(0, sc)
            add_half("ps_k", wk_sb, bk_sb, xfr_sb, kt_sb, 0, 1, 1)
            for sc in range(12, 14):
                proj_rest += vt_thunks(0, sc)
            add_half("ps_q", wq_sb, bq_sb, xto_sb, qt_sb, 0, 0, 1)
            for sc in range(14, 16):
                proj_rest += vt_thunks(0, sc)
            add_half("ps_q", wq_sb, bq_sb, xto_sb, qt_sb, 0, 1, 0)
            add_half("ps_q", wq_sb, bq_sb, xto_sb, qt_sb, 0, 1, 1)
            # remaining hp1 work (K tt0 / V^T 0-3 / Q h0 ran in the prefix)
            for tt_half in ((1, 0), (1, 1)):
                add_half(
                    "ps_k", wk_sb, bk_sb, xfr_sb, kt_sb, 1, tt_half[0],
                    tt_half[1],
                )
            for sc in range(4, 16):
                proj_rest += vt_thunks(1, sc)
            add_half("ps_q", wq_sb, bq_sb, xto_sb, qt_sb, 1, 0, 1)
            add_half("ps_q", wq_sb, bq_sb, xto_sb, qt_sb, 1, 1, 0)
            # Q(1,1) h1 is held out of proj_rest: its halves ride with the
            # outproj pushes in stripes 5/6 (their deadlines are that late)
            # so those stripes' boundary pops still find ready PE work.
            q11h1, _ = proj_half_thunks(
                "ps_q", wq_sb, bq_sb, xto_sb, qt_sb, 1, 1, 1
            )

            filler_pushes = {0: (proj_rest, 0, 3.5)}
            filler_pushes[5] = (
                q11h1 + emit_outproj_thunks(0), 5, 1.4,
            )
            filler_pushes[6] = (emit_outproj_thunks(1), 5, 1.0)
            filler_pushes[7] = (emit_outproj_thunks(2), 5, 1.0)
            emit_stripes(filler_pushes)
            for f in emit_outproj_thunks(3, store_on_act=True):
                f()

    nc.compile()
    return nc


def _prep_in_maps(x_to, x_from, Wq, bq, Wk, bk, Wv, bv, Wo):
    scale = 1.0 / np.sqrt(np.float32(DH))
    # [H, D, DH] -> [D, H*DH] with column h*DH+d
    wq_f = np.ascontiguousarray(Wq.transpose(1, 0, 2).reshape(D, H * DH)) * scale
    wk_f = np.ascontiguousarray(Wk.transpose(1, 0, 2).reshape(D, H * DH))
    wv_f = np.ascontiguousarray(Wv.transpose(1, 0, 2).reshape(D, H * DH))
    bq_f = bq.reshape(H * DH) * scale
    bk_f = bk.reshape(H * DH)
    bv_f = bv.reshape(H * DH)

    xt_to = np.ascontiguousarray(x_to.transpose(0, 2, 1))    # [B, D, TQ]
    xt_from = np.ascontiguousarray(x_from.transpose(0, 2, 1))

    def f32(a):
        return np.ascontiguousarray(a, dtype=np.float32)

    import ml_dtypes

    def fdt(a):
        return np.ascontiguousarray(a, dtype=ml_dtypes.bfloat16)

    in_maps = []
    for c in range(N_CORES):
        b, g = divmod(c, HEADS_PER_CORE)
        cs = slice(g * 256, (g + 1) * 256)
        in_maps.append(
            {
                "xt_to": fdt(xt_to[b]),
                "xt_from": fdt(xt_from[b]),
                "wq": fdt(wq_f[:, cs]),
                "wk": fdt(wk_f[:, cs]),
                "wv": fdt(wv_f[:, cs]),
                # [256] -> [2 pairs, 128] -> [128, 2]
                "bq": f32(bq_f[cs].reshape(2, 128).T),
                "bk": f32(bk_f[cs].reshape(2, 128).T),
                "bvt": fdt(bv_f[cs].reshape(1, 256)),
                # Wo[:, cs].T = [256, 1024] -> [2, 128, 1024] -> [128, 2, 1024]
                "wot": fdt(
                    np.ascontiguousarray(Wo[:, cs].T)
                    .reshape(2, 128, 1024)
                    .transpose(1, 0, 2)
                ),
            }
        )
    return in_maps


LAST_EXEC_TIME_NS = None
LAST_TRACE = None


def kernel(x_to, x_from, Wq, bq, Wk, bk, Wv, bv, Wo, bo):
    global LAST_EXEC_TIME_NS, LAST_TRACE
    if "nc" not in _CACHED:
        _CACHED["nc"] = build_program()
    nc = _CACHED["nc"]

    in_maps = _prep_in_maps(
        np.asarray(x_to), np.asarray(x_from), np.asarray(Wq), np.asarray(bq),
        np.asarray(Wk), np.asarray(bk), np.asarray(Wv), np.asarray(bv),
        np.asarray(Wo),
    )
    res = run_bass_kernel_spmd(nc, in_maps, list(range(N_CORES)))
    LAST_EXEC_TIME_NS = res.exec_time_ns
    LAST_TRACE = res.instructions_and_trace

    out = np.zeros((B, TQ, D), dtype=np.float32)
    for c in range(N_CORES):
        out[c // HEADS_PER_CORE] += res.results[c]["out"].astype(np.float32)
    out += np.asarray(bo, dtype=np.float32)
    return out
